# revision 7
# baseline (speedup 1.0000x reference)
"""DiffAttention Trainium2 kernel, 8-core SPMD (head-parallel).

Problem (hardcoded): B=2, S=2048, D=128, H=8.
  q = (x@Wq.T+bq).reshape(B,H,S,2D)   # raw reshape: head h <-> rows [256h,256h+256) of proj
  s1 = q1@k1.T; s2 = q2@k2.T; attn = softmax(s1) - lam*softmax(s2)
  out = attn@v -> transpose/reshape -> GroupNorm(H groups) -> *(1-lam) -> concat heads -> @Wo.T+bo

Sharding: core c owns head h=c for both batches (2 units/core). GroupNorm groups
mix all heads -> tiny (32-float) AllGather of partial stats.

Index algebra per unit (b,h), block = proj rows [256h, 256h+256):
  sigma (attn row) = 8r+j, r in [0,256), j in [0,8). We use tau-order sigma' = 256j+r.
  q1T[d, sigma'=256j+r] = qpT_block[f=256j+d, r]   (even 128-col chunks of qp block)
  q2T: odd chunks.  v'[sigma'=256j+r, d] = vp_block[r, 128j+d].
  GroupNorm group g = {sigma': (sigma' mod 256)//32 == g} (32-wide strips).
  Final rows: out[b, 8*rho+h, 128h3+d] = GN(O)[b,h][sigma'=256(rho%8)+32h3+rho//8, d]
"""

import sys

sys.path.insert(0, "/opt/trn_rl_repo")

import numpy as np

import concourse.bass as bass
import concourse.bacc as bacc
import concourse.mybir as mybir
import concourse.tile as tile
from concourse.bass_utils import run_bass_kernel_spmd

F32 = mybir.dt.float32
AF = mybir.ActivationFunctionType
ALU = mybir.AluOpType

B, S, D, H = 2, 2048, 128, 8
N_CORES = 8
EPS = 1e-5
GROUP_N = float(256 * H * D)  # elements per GroupNorm group

_CACHED = None


def build_nc():
    nc = bacc.Bacc("TRN2", target_bir_lowering=False, debug=False, num_devices=N_CORES)

    # ---- per-core external I/O ----
    qT = nc.dram_tensor("qT", [B, 128, 256], F32, kind="ExternalInput")  # query block.T per batch
    wqT = nc.dram_tensor("wqT", [128, 2048], F32, kind="ExternalInput")
    wkT = nc.dram_tensor("wkT", [128, 2048], F32, kind="ExternalInput")
    wvT = nc.dram_tensor("wvT", [128, 1024], F32, kind="ExternalInput")
    woT = nc.dram_tensor("woT", [1024, 128], F32, kind="ExternalInput")
    bqT = nc.dram_tensor("bqT", [128, 16], F32, kind="ExternalInput")
    bkT = nc.dram_tensor("bkT", [128, 16], F32, kind="ExternalInput")
    bv = nc.dram_tensor("bv", [1, 1024], F32, kind="ExternalInput")
    bo = nc.dram_tensor("bo", [1, 128], F32, kind="ExternalInput")
    gnw2 = nc.dram_tensor("gnw2", [1, 16], F32, kind="ExternalInput")  # tiled x2 (b,g)
    gnb2 = nc.dram_tensor("gnb2", [1, 16], F32, kind="ExternalInput")
    lam = nc.dram_tensor("lam", [1, 1], F32, kind="ExternalInput")
    outp = nc.dram_tensor("outp", [B, 256, 128], F32, kind="ExternalOutput")

    with tile.TileContext(nc) as tc:
        with (
            tc.tile_pool(name="const", bufs=1) as cpool,
            tc.tile_pool(name="proj", bufs=2) as projpool,
            tc.tile_pool(name="vpool", bufs=4) as vpool,
            tc.tile_pool(name="epool", bufs=3) as epool,
            tc.tile_pool(name="otpool", bufs=2) as otpool,
            tc.tile_pool(name="tmp", bufs=2) as tmppool,
            tc.tile_pool(name="ps_s", bufs=2, space="PSUM") as ps_s,
            tc.tile_pool(name="ps_acc", bufs=1, space="PSUM") as ps_acc,
            tc.tile_pool(name="dram", bufs=1, space="DRAM") as dram,
        ):
            # ---- load constants / weights ----
            wq_sb = cpool.tile([128, 2048], F32)
            wk_sb = cpool.tile([128, 2048], F32)
            wv_sb = cpool.tile([128, 1024], F32)
            nc.sync.dma_start(wq_sb[:], wqT[:])
            nc.sync.dma_start(wk_sb[:], wkT[:])
            nc.sync.dma_start(wv_sb[:], wvT[:])
            wo_sb = []
            for h3 in range(8):
                w = cpool.tile([128, 128], F32, name=f"wo_sb{h3}")
                nc.sync.dma_start(w[:], woT[128 * h3 : 128 * (h3 + 1), :])
                wo_sb.append(w)
            bq_sb = cpool.tile([128, 16], F32)
            bk_sb = cpool.tile([128, 16], F32)
            nc.sync.dma_start(bq_sb[:], bqT[:])
            nc.sync.dma_start(bk_sb[:], bkT[:])
            bv_sb = cpool.tile([1, 1024], F32)
            nc.sync.dma_start(bv_sb[:], bv[:])
            bo_sb = cpool.tile([1, 128], F32)
            nc.sync.dma_start(bo_sb[:], bo[:])
            gnw_sb = cpool.tile([1, 16], F32)
            gnb_sb = cpool.tile([1, 16], F32)
            nc.sync.dma_start(gnw_sb[:], gnw2[:])
            nc.sync.dma_start(gnb_sb[:], gnb2[:])
            lam_sb = cpool.tile([1, 1], F32)
            nc.sync.dma_start(lam_sb[:], lam[:])
            qt_sb = []
            for u in range(B):
                q = cpool.tile([128, 256], F32, name=f"qt_sb{u}")
                nc.sync.dma_start(q[:], qT[u])
                qt_sb.append(q)

            ones_sb = cpool.tile([128, 128], F32)
            nc.vector.memset(ones_sb[:], 1.0)
            lam_rep = cpool.tile([128, 1], F32)
            nc.gpsimd.partition_broadcast(lam_rep[:], lam_sb[:])
            bv_rep = cpool.tile([128, 1024], F32)
            nc.gpsimd.partition_broadcast(bv_rep[:], bv_sb[:])
            bo_rep = cpool.tile([128, 128], F32)
            nc.gpsimd.partition_broadcast(bo_rep[:], bo_sb[:])

            # ================= per-unit projections + attention =================
            ot_sb = []  # OT [128 d, 2048 sigma'] per unit
            for u in range(B):
                q1t = projpool.tile([128, 2048], F32, tag="q1t", name=f"q1t_{u}")
                q2t = projpool.tile([128, 2048], F32, tag="q2t", name=f"q2t_{u}")
                k1t = projpool.tile([128, 2048], F32, tag="k1t", name=f"k1t_{u}")
                k2t = projpool.tile([128, 2048], F32, tag="k2t", name=f"k2t_{u}")
                # qp/kp transposed chunks: [f 128, r 256] = W.T_chunk.T @ queryT_block
                for (wsb, bsb, d1, d2) in ((wq_sb, bq_sb, q1t, q2t), (wk_sb, bk_sb, k1t, k2t)):
                    for j in range(16):
                        ps = ps_s.tile([128, 256], F32, tag="s", name=f"pp_{u}_{j}")
                        nc.tensor.matmul(
                            ps[:], wsb[:, 128 * j : 128 * (j + 1)], qt_sb[u][:],
                            start=True, stop=True,
                        )
                        dst = d1 if j % 2 == 0 else d2
                        col = 256 * (j // 2)
                        # bias add fused into PSUM->SBUF copy (per-partition scalar)
                        nc.vector.tensor_scalar_add(
                            dst[:, col : col + 256], ps[:], bsb[:, j : j + 1]
                        )
                # vp natural: [r' 128, f 1024] x2 row-chunks
                vp = []
                for rc in range(2):
                    vt = vpool.tile([128, 1024], F32, tag="vp", name=f"vp_{u}_{rc}")
                    for fh in range(2):
                        ps = ps_s.tile([128, 512], F32, tag="s", name=f"ppv_{u}_{rc}_{fh}")
                        nc.tensor.matmul(
                            ps[:], qt_sb[u][:, 128 * rc : 128 * (rc + 1)],
                            wv_sb[:, 512 * fh : 512 * (fh + 1)],
                            start=True, stop=True,
                        )
                        nc.vector.tensor_tensor(
                            vt[:, 512 * fh : 512 * (fh + 1)], ps[:],
                            bv_rep[:, 512 * fh : 512 * (fh + 1)], ALU.add,
                        )
                    vp.append(vt)

                def vchunk(kc):
                    # v' chunk kc: [128 k, 128 d] = vp[kc%2][:, 128*(kc//2):+128]
                    return vp[kc % 2][:, 128 * (kc // 2) : 128 * (kc // 2) + 128]

                ot = otpool.tile([128, 2048], F32, tag="ot", name=f"ot_{u}")
                ot_sb.append(ot)

                for qb in range(4):
                    qsl = slice(512 * qb, 512 * (qb + 1))
                    u1 = ps_acc.tile([128, 512], F32, tag="u1", name=f"u1_{u}_{qb}")
                    u2 = ps_acc.tile([128, 512], F32, tag="u2", name=f"u2_{u}_{qb}")
                    r1 = ps_acc.tile([128, 512], F32, tag="r1", name=f"r1_{u}_{qb}")
                    r2 = ps_acc.tile([128, 512], F32, tag="r2", name=f"r2_{u}_{qb}")
                    for kcg in range(8):
                        for m, (kt, qt_, uacc, racc) in enumerate(
                            ((k1t, q1t, u1, r1), (k2t, q2t, u2, r2))
                        ):
                            sgrp = ps_s.tile([128, 1024], F32, tag="s", name=f"s_{u}_{qb}_{kcg}_{m}")
                            for h in range(2):
                                kc = 2 * kcg + h
                                nc.tensor.matmul(
                                    sgrp[:, 512 * h : 512 * (h + 1)],
                                    kt[:, 128 * kc : 128 * (kc + 1)],
                                    qt_[:, qsl],
                                    start=True, stop=True,
                                )
                            eg = epool.tile([128, 1024], F32, tag="e", name=f"e_{u}_{qb}_{kcg}_{m}")
                            nc.scalar.activation(eg[:], sgrp[:], AF.Exp)
                            for h in range(2):
                                kc = 2 * kcg + h
                                esl = eg[:, 512 * h : 512 * (h + 1)]
                                nc.tensor.matmul(
                                    racc[:], ones_sb[:], esl,
                                    start=(kcg == 0 and h == 0),
                                    stop=(kcg == 7 and h == 1),
                                )
                                nc.tensor.matmul(
                                    uacc[:], vchunk(kc), esl,
                                    start=(kcg == 0 and h == 0),
                                    stop=(kcg == 7 and h == 1),
                                )
                    # O = U1/R1 - lam*U2/R2   (R replicated across partitions)
                    r1i = tmppool.tile([128, 512], F32, tag="r1i", name=f"r1i_{u}_{qb}")
                    r2i = tmppool.tile([128, 512], F32, tag="r2i", name=f"r2i_{u}_{qb}")
                    nc.vector.reciprocal(r1i[:], r1[:])
                    nc.vector.reciprocal(r2i[:], r2[:])
                    t2 = tmppool.tile([128, 512], F32, tag="t2", name=f"t2_{u}_{qb}")
                    nc.vector.scalar_tensor_tensor(
                        t2[:], u2[:], lam_rep[:, 0:1], r2i[:], ALU.mult, ALU.mult
                    )
                    t1 = tmppool.tile([128, 512], F32, tag="t1", name=f"t1_{u}_{qb}")
                    nc.vector.tensor_tensor(t1[:], u1[:], r1i[:], ALU.mult)
                    nc.vector.tensor_tensor(ot[:, qsl], t1[:], t2[:], ALU.subtract)

            # ================= GroupNorm stats + AllGather =================
            stats_sb = tmppool.tile([1, 32], F32, tag="stats")
            for u in range(B):
                sq = tmppool.tile([128, 2048], F32, tag="sq", name=f"sq_{u}")
                nc.vector.tensor_tensor(sq[:], ot_sb[u][:], ot_sb[u][:], ALU.mult)
                for si, src in enumerate((ot_sb[u], sq)):
                    p1 = tmppool.tile([128, 64], F32, tag="p1", name=f"p1_{u}_{si}")
                    v = src.rearrange("p (j g r) -> p j g r", j=8, g=8, r=32)
                    nc.vector.tensor_reduce(p1[:], v, mybir.AxisListType.X, ALU.add)
                    p2 = tmppool.tile([128, 8], F32, tag="p2", name=f"p2_{u}_{si}")
                    nc.vector.tensor_reduce(
                        p2[:], p1.rearrange("p (j g) -> p g j", j=8, g=8),
                        mybir.AxisListType.X, ALU.add,
                    )
                    st = ps_s.tile([1, 8], F32, tag="s", name=f"st_{u}_{si}")
                    nc.tensor.matmul(st[:], ones_sb[:, 0:1], p2[:], start=True, stop=True)
                    nc.vector.tensor_copy(
                        stats_sb[:, 16 * si + 8 * u : 16 * si + 8 * u + 8], st[:]
                    )
            cc_in = dram.tile([1, 32], F32)
            cc_out = dram.tile([8, 32], F32, addr_space="Shared")
            nc.sync.dma_start(cc_in[:], stats_sb[:])
            nc.gpsimd.collective_compute(
                "AllGather", ALU.bypass,
                replica_groups=[list(range(N_CORES))],
                ins=[cc_in[:]], outs=[cc_out[:]],
            )
            gath = tmppool.tile([8, 32], F32, tag="gath")
            nc.sync.dma_start(gath[:], cc_out[:])
            glob_ps = ps_s.tile([1, 32], F32, tag="s", name="glob_ps")
            nc.tensor.matmul(glob_ps[:], ones_sb[0:8, 0:1], gath[:], start=True, stop=True)
            glob = tmppool.tile([1, 32], F32, tag="globsb")
            nc.vector.tensor_copy(glob[:], glob_ps[:])
            # [1,32] layout: [sum b0(8) | sum b1(8) | sq b0(8) | sq b1(8)]
            sums = glob[:, 0:16]
            sqs = glob[:, 16:32]
            mean = tmppool.tile([1, 16], F32, tag="mean")
            ex2 = tmppool.tile([1, 16], F32, tag="ex2")
            nc.vector.tensor_scalar_mul(mean[:], sums, 1.0 / GROUP_N)
            nc.vector.tensor_scalar_mul(ex2[:], sqs, 1.0 / GROUP_N)
            var = tmppool.tile([1, 16], F32, tag="var")
            nc.vector.tensor_tensor(var[:], mean[:], mean[:], ALU.mult)
            nc.vector.tensor_tensor(var[:], ex2[:], var[:], ALU.subtract)
            veps = tmppool.tile([1, 16], F32, tag="veps")
            nc.vector.tensor_scalar_add(veps[:], var[:], EPS)
            rstd = tmppool.tile([1, 16], F32, tag="rstd")
            vinv = tmppool.tile([1, 16], F32, tag="vinv")
            nc.vector.reciprocal(vinv[:], veps[:])
            nc.scalar.activation(rstd[:], vinv[:], AF.Sqrt)
            # one Newton step: rstd *= 1.5 - 0.5*veps*rstd^2
            nt = tmppool.tile([1, 16], F32, tag="nt")
            nc.vector.tensor_tensor(nt[:], veps[:], rstd[:], ALU.mult)
            nc.vector.tensor_tensor(nt[:], nt[:], rstd[:], ALU.mult)
            nc.scalar.activation(nt[:], nt[:], AF.Copy, bias=1.5, scale=-0.5)
            nc.vector.tensor_tensor(rstd[:], rstd[:], nt[:], ALU.mult)
            # A = rstd*gnw*(1-lam); Bc = (gnb - mean*rstd*gnw)*(1-lam)
            oml = tmppool.tile([1, 1], F32, tag="oml")
            nc.vector.tensor_scalar(oml[:], lam_sb[:], -1.0, 1.0, ALU.mult, ALU.add)
            A = tmppool.tile([1, 16], F32, tag="A")
            nc.vector.tensor_tensor(A[:], rstd[:], gnw_sb[:], ALU.mult)
            Bc = tmppool.tile([1, 16], F32, tag="Bc")
            nc.vector.tensor_tensor(Bc[:], mean[:], A[:], ALU.mult)
            nc.vector.tensor_tensor(Bc[:], gnb_sb[:], Bc[:], ALU.subtract)
            nc.vector.tensor_scalar_mul(A[:], A[:], oml[:, 0:1])
            nc.vector.tensor_scalar_mul(Bc[:], Bc[:], oml[:, 0:1])
            A_rep = tmppool.tile([128, 16], F32, tag="A_rep")
            B_rep = tmppool.tile([128, 16], F32, tag="B_rep")
            nc.gpsimd.partition_broadcast(A_rep[:], A[:])
            nc.gpsimd.partition_broadcast(B_rep[:], Bc[:])

            # ================= GN apply (fused re-layout) + output matmul =================
            # fT[d, 256*h3 + 32*j + r] = GN(OT)[d, 256*j + 32*h3 + r]
            # -> fT chunk h3 = F_h.T[(h3,d) block] with column m' = 32*(rho%8) + rho//8
            for u in range(B):
                otv = ot_sb[u].rearrange("p (j g r) -> p g j r", j=8, g=8, r=32)
                fT = tmppool.tile([128, 2048], F32, tag="sq", name=f"fT_{u}")
                fv = fT.rearrange("p (g j r) -> p g j r", g=8, j=8, r=32)
                for g in range(8):
                    c = 8 * u + g
                    nc.vector.tensor_scalar(
                        fv[:, g], otv[:, g],
                        A_rep[:, c : c + 1], B_rep[:, c : c + 1],
                        ALU.mult, ALU.add,
                    )
                # res chunk rh: partition m'' holds row rho = 8*(m''%32) + 4*rh + m''//32
                for rh in range(2):
                    res = ps_s.tile([128, 128], F32, tag="s", name=f"res_{u}_{rh}")
                    for h3 in range(8):
                        lhsT = fT[:, 256 * h3 + 128 * rh : 256 * h3 + 128 * rh + 128]
                        nc.tensor.matmul(
                            res[:], lhsT, wo_sb[h3][:],
                            start=(h3 == 0), stop=(h3 == 7),
                        )
                    rsb = tmppool.tile([128, 128], F32, tag="rsb", name=f"rsb_{u}_{rh}")
                    nc.vector.tensor_tensor(rsb[:], res[:], bo_rep[:], ALU.add)
                    dst = outp[u].rearrange("(b rh c) d -> rh c b d", b=32, rh=2, c=4)
                    nc.sync.dma_start(dst[rh], rsb[:])

    nc.compile()
    return nc


def _prep_inputs(inputs):
    """Host-side: slice/transpose full inputs into per-core in_maps."""
    query = np.asarray(inputs["query"], np.float32)
    Wq = np.asarray(inputs["Wq"], np.float32)
    Wk = np.asarray(inputs["Wk"], np.float32)
    Wv = np.asarray(inputs["Wv"], np.float32)
    Wo = np.asarray(inputs["Wo"], np.float32)
    bq = np.asarray(inputs["bq"], np.float32)
    bk = np.asarray(inputs["bk"], np.float32)
    bv = np.asarray(inputs["bv"], np.float32)
    bo = np.asarray(inputs["bo"], np.float32)
    gn_w = np.asarray(inputs["gn_w"], np.float32)
    gn_b = np.asarray(inputs["gn_b"], np.float32)
    lam = np.asarray(inputs["lam"], np.float32).reshape(1, 1)

    shared = {
        "wqT": np.ascontiguousarray(Wq.T),
        "wkT": np.ascontiguousarray(Wk.T),
        "wvT": np.ascontiguousarray(Wv.T),
        "woT": np.ascontiguousarray(Wo.T),
        "bqT": np.ascontiguousarray(bq.reshape(16, 128).T),
        "bkT": np.ascontiguousarray(bk.reshape(16, 128).T),
        "bv": bv.reshape(1, 1024),
        "bo": bo.reshape(1, 128),
        "gnw2": np.tile(gn_w, 2).reshape(1, 16),
        "gnb2": np.tile(gn_b, 2).reshape(1, 16),
        "lam": lam,
    }
    in_maps = []
    for c in range(N_CORES):
        blk = query[:, 256 * c : 256 * (c + 1), :]  # [B, 256, 128]
        qT = np.ascontiguousarray(blk.transpose(0, 2, 1))  # [B, 128, 256]
        in_maps.append({"qT": qT, **shared})
    return in_maps


def kernel(**inputs) -> np.ndarray:
    global _CACHED
    if _CACHED is None:
        _CACHED = build_nc()
    nc = _CACHED
    in_maps = _prep_inputs(inputs)
    res = run_bass_kernel_spmd(nc, in_maps, core_ids=list(range(N_CORES)))
    out = np.empty((B, S, H * D // 8), np.float32)  # (2, 2048, 128)
    for c in range(N_CORES):
        o = res.results[c]["outp"]  # [B, 256, 128]
        for b in range(B):
            out[b, c::8, :] = o[b]  # rows s3 = 8*rho + c
    return out


# revision 21
# speedup vs baseline: 3458.9095x; 3458.9095x over previous
"""DiffAttention Trainium2 kernel, 8-core SPMD (head-parallel).

Problem (hardcoded): B=2, S=2048, D=128, H=8.
  q = (x@Wq.T+bq).reshape(B,H,S,2D)   # raw reshape: head h <-> rows [256h,256h+256) of proj
  s1 = q1@k1.T; s2 = q2@k2.T; attn = softmax(s1) - lam*softmax(s2)
  out = attn@v -> transpose/reshape -> GroupNorm(H groups) -> *(1-lam) -> concat heads -> @Wo.T+bo

Sharding: core c owns head h=c for both batches (2 units/core). GroupNorm groups
mix all heads -> tiny (32-float) AllGather of partial stats.

Index algebra per unit (b,h), block = proj rows [256h, 256h+256):
  sigma (attn row) = 8r+j, r in [0,256), j in [0,8). We use tau-order sigma' = 256j+r.
  q1T[d, sigma'=256j+r] = qpT_block[f=256j+d, r]   (even 128-col chunks of qp block)
  q2T: odd chunks.  v'[sigma'=256j+r, d] = vp_block[r, 128j+d].
  GroupNorm group g = {sigma': (sigma' mod 256)//32 == g} (32-wide strips).
  Final rows: out[b, 8*rho+h, 128h3+d] = GN(O)[b,h][sigma'=256(rho%8)+32h3+rho//8, d]
"""

import sys

sys.path.insert(0, "/opt/trn_rl_repo")

import numpy as np

import concourse.bass as bass
import concourse.bacc as bacc
import concourse.mybir as mybir
import concourse.tile as tile
from concourse.bass_utils import run_bass_kernel_spmd

F32 = mybir.dt.float32
F32R = mybir.dt.float32r
AF = mybir.ActivationFunctionType
ALU = mybir.AluOpType

B, S, D, H = 2, 2048, 128, 8
N_CORES = 8
EPS = 1e-5
GROUP_N = float(256 * H * D)  # elements per GroupNorm group

_CACHED = None


def build_nc():
    nc = bacc.Bacc("TRN2", target_bir_lowering=False, debug=False, num_devices=N_CORES)

    # ---- per-core external I/O ----
    qT = nc.dram_tensor("qT", [B, 128, 256], F32, kind="ExternalInput")  # query block.T per batch
    wqT = nc.dram_tensor("wqT", [128, 2048], F32, kind="ExternalInput")
    wkT = nc.dram_tensor("wkT", [128, 2048], F32, kind="ExternalInput")
    wvT = nc.dram_tensor("wvT", [128, 1024], F32, kind="ExternalInput")
    woT = nc.dram_tensor("woT", [1024, 128], F32, kind="ExternalInput")
    bqT = nc.dram_tensor("bqT", [128, 16], F32, kind="ExternalInput")
    bkT = nc.dram_tensor("bkT", [128, 16], F32, kind="ExternalInput")
    bv = nc.dram_tensor("bv", [1, 1024], F32, kind="ExternalInput")
    bo = nc.dram_tensor("bo", [1, 128], F32, kind="ExternalInput")
    gnw2 = nc.dram_tensor("gnw2", [1, 16], F32, kind="ExternalInput")  # tiled x2 (b,g)
    gnb2 = nc.dram_tensor("gnb2", [1, 16], F32, kind="ExternalInput")
    lam = nc.dram_tensor("lam", [1, 1], F32, kind="ExternalInput")
    outp = nc.dram_tensor("outp", [B, 256, 128], F32, kind="ExternalOutput")

    with tile.TileContext(nc) as tc:
        with (
            tc.tile_pool(name="const", bufs=1) as cpool,
            tc.tile_pool(name="proj", bufs=2) as projpool,
            tc.tile_pool(name="vpool", bufs=4) as vpool,
            tc.tile_pool(name="epool", bufs=3) as epool,
            tc.tile_pool(name="otpool", bufs=2) as otpool,
            tc.tile_pool(name="tmp", bufs=2) as tmppool,
            tc.tile_pool(name="ps_s", bufs=2, space="PSUM") as ps_s,
            tc.tile_pool(name="ps_acc", bufs=1, space="PSUM") as ps_acc,
            tc.tile_pool(name="dram", bufs=1, space="DRAM") as dram,
        ):
            # ---- load constants / weights (qT first: projections need it) ----
            qt_sb = []
            for u in range(B):
                q = cpool.tile([128, 256], F32, name=f"qt_sb{u}")
                nc.sync.dma_start(q[:], qT[u])
                qt_sb.append(q)

            wo_sb = []
            for h3 in range(8):
                w = cpool.tile([128, 128], F32, name=f"wo_sb{h3}")
                nc.sync.dma_start(w[:], woT[128 * h3 : 128 * (h3 + 1), :])
                wo_sb.append(w)
            bq_sb = cpool.tile([128, 16], F32)
            bk_sb = cpool.tile([128, 16], F32)
            nc.sync.dma_start(bq_sb[:], bqT[:])
            nc.sync.dma_start(bk_sb[:], bkT[:])
            bv_sb = cpool.tile([1, 1024], F32)
            nc.sync.dma_start(bv_sb[:], bv[:])
            bo_sb = cpool.tile([1, 128], F32)
            nc.sync.dma_start(bo_sb[:], bo[:])
            gnw_sb = cpool.tile([1, 16], F32)
            gnb_sb = cpool.tile([1, 16], F32)
            nc.sync.dma_start(gnw_sb[:], gnw2[:])
            nc.sync.dma_start(gnb_sb[:], gnb2[:])
            lam_sb = cpool.tile([1, 1], F32)
            nc.sync.dma_start(lam_sb[:], lam[:])
            ones_f32 = cpool.tile([128, 128], F32)
            nc.vector.memset(ones_f32[:], 1.0)
            ones_sb = cpool.tile([128, 128], F32R)
            nc.vector.tensor_copy(ones_sb[:], ones_f32[:])
            wq_r = cpool.tile([128, 2048], F32R)
            wk_r = cpool.tile([128, 2048], F32R)
            wv_r = cpool.tile([128, 1024], F32R)
            for (dram_w, w_r, wtag) in (
                (wqT, wq_r, "q1t"), (wkT, wk_r, "q2t"), (wvT, wv_r, "k1t")
            ):
                wsc = projpool.tile([128, 2048], F32, tag=wtag, name=f"wsc_{wtag}")
                sl = wsc[:, 0 : w_r.shape[1]]
                nc.sync.dma_start(sl, dram_w[:])
                nc.vector.tensor_copy(w_r[:], sl)
            qt_r = []
            for u in range(B):
                qr = cpool.tile([128, 256], F32R, name=f"qt_r{u}")
                nc.vector.tensor_copy(qr[:], qt_sb[u][:])
                qt_r.append(qr)
            lam_rep = cpool.tile([128, 1], F32)
            nc.gpsimd.partition_broadcast(lam_rep[:], lam_sb[:])
            bv_rep = cpool.tile([128, 1024], F32)
            nc.gpsimd.partition_broadcast(bv_rep[:], bv_sb[:])
            bo_rep = cpool.tile([128, 128], F32)
            nc.gpsimd.partition_broadcast(bo_rep[:], bo_sb[:])
            # column sums of Wo chunks (for the GroupNorm beta term)
            wsum_sb = cpool.tile([1, 1024], F32)
            for h3 in range(8):
                wps = ps_s.tile([1, 128], F32, tag="s", name=f"wps_{h3}")
                nc.tensor.matmul(wps[:], ones_f32[:, 0:1], wo_sb[h3][:], start=True, stop=True)
                nc.vector.tensor_copy(wsum_sb[:, 128 * h3 : 128 * (h3 + 1)], wps[:])

            # ================= projections (both units) =================
            stats_sb = tmppool.tile([1, 32], F32, tag="stats")
            p2_tiles = []
            ot_sb = []  # OT [128 d, 2048 sigma'] per unit
            proj = []  # (q1t, q2t, k1t, k2t, vp) per unit
            for u in range(B):
                q1t = projpool.tile([128, 2048], F32R, tag="q1t", name=f"q1t_{u}")
                q2t = projpool.tile([128, 2048], F32R, tag="q2t", name=f"q2t_{u}")
                k1t = projpool.tile([128, 2048], F32R, tag="k1t", name=f"k1t_{u}")
                k2t = projpool.tile([128, 2048], F32R, tag="k2t", name=f"k2t_{u}")
                # qp/kp transposed chunks: [f 128, r 256] = W.T_chunk.T @ queryT_block
                for (wsb, bsb, d1, d2) in ((wq_r, bq_sb, q1t, q2t), (wk_r, bk_sb, k1t, k2t)):
                    for j in range(16):
                        ps = ps_s.tile([128, 256], F32, tag="s", name=f"pp_{u}_{j}")
                        nc.tensor.matmul(
                            ps[:], wsb[:, 128 * j : 128 * (j + 1)], qt_r[u][:],
                            start=True, stop=True,
                        )
                        dst = d1 if j % 2 == 0 else d2
                        col = 256 * (j // 2)
                        # bias add fused into PSUM->SBUF copy (per-partition scalar)
                        nc.vector.tensor_scalar_add(
                            dst[:, col : col + 256], ps[:], bsb[:, j : j + 1]
                        )
                # vp natural: [r' 128, f 1024] x2 row-chunks
                vp = []
                for rc in range(2):
                    vt = vpool.tile([128, 1024], F32R, tag="vp", name=f"vp_{u}_{rc}")
                    for fh in range(2):
                        ps = ps_s.tile([128, 512], F32, tag="s", name=f"ppv_{u}_{rc}_{fh}")
                        nc.tensor.matmul(
                            ps[:], qt_r[u][:, 128 * rc : 128 * (rc + 1)],
                            wv_r[:, 512 * fh : 512 * (fh + 1)],
                            start=True, stop=True,
                        )
                        nc.vector.tensor_tensor(
                            vt[:, 512 * fh : 512 * (fh + 1)], ps[:],
                            bv_rep[:, 512 * fh : 512 * (fh + 1)], ALU.add,
                        )
                    vp.append(vt)
                proj.append((q1t, q2t, k1t, k2t, vp))

            # ================= attention + stats (both units) =================
            for u in range(B):
                q1t, q2t, k1t, k2t, vp = proj[u]

                def vchunk(kc):
                    # v' chunk kc: [128 k, 128 d] = vp[kc%2][:, 128*(kc//2):+128]
                    return vp[kc % 2][:, 128 * (kc // 2) : 128 * (kc // 2) + 128]

                ot = otpool.tile([128, 2048], F32, tag="ot", name=f"ot_{u}")
                ot_sb.append(ot)

                for qb in range(4):
                    qsl = slice(512 * qb, 512 * (qb + 1))
                    u1 = ps_acc.tile([128, 512], F32, tag="u1", name=f"u1_{u}_{qb}")
                    u2 = ps_acc.tile([128, 512], F32, tag="u2", name=f"u2_{u}_{qb}")
                    r1 = ps_acc.tile([128, 512], F32, tag="r1", name=f"r1_{u}_{qb}")
                    r2 = ps_acc.tile([128, 512], F32, tag="r2", name=f"r2_{u}_{qb}")
                    def consume(item):
                        kcg, eg, uacc, racc = item
                        for h in range(2):
                            kc = 2 * kcg + h
                            esl = eg[:, 512 * h : 512 * (h + 1)]
                            nc.tensor.matmul(
                                racc[:], ones_sb[:], esl,
                                start=(kcg == 0 and h == 0),
                                stop=(kcg == 7 and h == 1),
                            )
                            nc.tensor.matmul(
                                uacc[:], vchunk(kc), esl,
                                start=(kcg == 0 and h == 0),
                                stop=(kcg == 7 and h == 1),
                            )

                    pending = None
                    for kcg in range(8):
                        for m, (kt, qt_, uacc, racc) in enumerate(
                            ((k1t, q1t, u1, r1), (k2t, q2t, u2, r2))
                        ):
                            sgrp = ps_s.tile([128, 1024], F32, tag="s", name=f"s_{u}_{qb}_{kcg}_{m}")
                            for h in range(2):
                                kc = 2 * kcg + h
                                nc.tensor.matmul(
                                    sgrp[:, 512 * h : 512 * (h + 1)],
                                    kt[:, 128 * kc : 128 * (kc + 1)],
                                    qt_[:, qsl],
                                    start=True, stop=True,
                                )
                            eg = epool.tile([128, 1024], F32R, tag="e", name=f"e_{u}_{qb}_{kcg}_{m}")
                            nc.scalar.activation(eg[:], sgrp[:], AF.Exp)
                            if pending is not None:
                                consume(pending)
                            pending = (kcg, eg, uacc, racc)
                    consume(pending)
                    # O = U1/R1 - lam*U2/R2   (R replicated across partitions)
                    r1i = tmppool.tile([128, 512], F32, tag="r1i", name=f"r1i_{u}_{qb}")
                    r2i = tmppool.tile([128, 512], F32, tag="r2i", name=f"r2i_{u}_{qb}")
                    nc.vector.reciprocal(r1i[:], r1[:])
                    nc.vector.reciprocal(r2i[:], r2[:])
                    t2 = tmppool.tile([128, 512], F32, tag="t2", name=f"t2_{u}_{qb}")
                    nc.vector.scalar_tensor_tensor(
                        t2[:], u2[:], lam_rep[:, 0:1], r2i[:], ALU.mult, ALU.mult
                    )
                    t1 = tmppool.tile([128, 512], F32, tag="t1", name=f"t1_{u}_{qb}")
                    nc.vector.tensor_tensor(t1[:], u1[:], r1i[:], ALU.mult)
                    nc.vector.tensor_tensor(ot[:, qsl], t1[:], t2[:], ALU.subtract)

                # per-unit GroupNorm partial stats on DVE (overlaps next unit's
                # attention); the tiny PE partition-sum matmuls are deferred so
                # the in-order PE queue never waits on this DVE chain.
                sq = tmppool.tile([128, 2048], F32, tag="sq", name=f"sq_{u}")
                nc.vector.tensor_tensor(sq[:], ot[:], ot[:], ALU.mult)
                for si, ssrc in enumerate((ot, sq)):
                    p1 = tmppool.tile([128, 64], F32, tag="p1", name=f"p1_{u}_{si}")
                    v = ssrc.rearrange("p (j g r) -> p j g r", j=8, g=8, r=32)
                    nc.vector.tensor_reduce(p1[:], v, mybir.AxisListType.X, ALU.add)
                    p2 = tmppool.tile([128, 8], F32, tag="p2", name=f"p2_{u}_{si}")
                    nc.vector.tensor_reduce(
                        p2[:], p1.rearrange("p (j g) -> p g j", j=8, g=8),
                        mybir.AxisListType.X, ALU.add,
                    )
                    p2_tiles.append((u, si, p2))

            # deferred partition-sums of the per-unit stats
            for (u, si, p2) in p2_tiles:
                st = ps_s.tile([1, 8], F32, tag="s", name=f"st_{u}_{si}")
                nc.tensor.matmul(st[:], ones_f32[:, 0:1], p2[:], start=True, stop=True)
                nc.vector.tensor_copy(
                    stats_sb[:, 16 * si + 8 * u : 16 * si + 8 * u + 8], st[:]
                )

            # ================= AllGather of per-unit stats =================
            cc_in = dram.tile([1, 32], F32)
            cc_out = dram.tile([8, 32], F32, addr_space="Shared")
            nc.sync.dma_start(cc_in[:], stats_sb[:])
            nc.gpsimd.collective_compute(
                "AllGather", ALU.bypass,
                replica_groups=[list(range(N_CORES))],
                ins=[cc_in[:]], outs=[cc_out[:]],
            )

            # -- overlapped with the collective: re-layout OT and compute the
            #    unnormalized per-h3 partial output matmuls P[u][rh][h3] --
            # fT[d, 256g + 32j + r] = OT[d, 256j + 32g + r]
            P_sb = {}
            for u in range(B):
                fT = tmppool.tile([128, 2048], F32, tag="sq", name=f"fT_{u}")
                fv = fT.rearrange("p (g j r) -> p g j r", g=8, j=8, r=32)
                otv = ot_sb[u].rearrange("p (j g r) -> p g j r", j=8, g=8, r=32)
                for g in range(8):
                    nc.vector.tensor_copy(fv[:, g], otv[:, g])
                for rh in range(2):
                    for h3 in range(8):
                        pps = ps_s.tile([128, 128], F32, tag="s", name=f"pps_{u}_{rh}_{h3}")
                        lhsT = fT[:, 256 * h3 + 128 * rh : 256 * h3 + 128 * rh + 128]
                        nc.tensor.matmul(pps[:], lhsT, wo_sb[h3][:], start=True, stop=True)
                        pt = cpool.tile([128, 128], F32, name=f"P_{u}_{rh}_{h3}")
                        nc.vector.tensor_copy(pt[:], pps[:])
                        P_sb[(u, rh, h3)] = pt

            gath = tmppool.tile([8, 32], F32, tag="gath")
            nc.sync.dma_start(gath[:], cc_out[:])
            glob_ps = ps_s.tile([1, 32], F32, tag="s", name="glob_ps")
            nc.tensor.matmul(glob_ps[:], ones_f32[0:8, 0:1], gath[:], start=True, stop=True)
            glob = tmppool.tile([1, 32], F32, tag="globsb")
            nc.vector.tensor_copy(glob[:], glob_ps[:])
            # [1,32] layout: [sum b0(8) | sum b1(8) | sq b0(8) | sq b1(8)]
            sums = glob[:, 0:16]
            sqs = glob[:, 16:32]
            mean = tmppool.tile([1, 16], F32, tag="mean")
            ex2 = tmppool.tile([1, 16], F32, tag="ex2")
            nc.vector.tensor_scalar_mul(mean[:], sums, 1.0 / GROUP_N)
            nc.vector.tensor_scalar_mul(ex2[:], sqs, 1.0 / GROUP_N)
            var = tmppool.tile([1, 16], F32, tag="var")
            nc.vector.tensor_tensor(var[:], mean[:], mean[:], ALU.mult)
            nc.vector.tensor_tensor(var[:], ex2[:], var[:], ALU.subtract)
            veps = tmppool.tile([1, 16], F32, tag="veps")
            nc.vector.tensor_scalar_add(veps[:], var[:], EPS)
            rstd = tmppool.tile([1, 16], F32, tag="rstd")
            vinv = tmppool.tile([1, 16], F32, tag="vinv")
            nc.vector.reciprocal(vinv[:], veps[:])
            nc.scalar.activation(rstd[:], vinv[:], AF.Sqrt)
            # one Newton step: rstd *= 1.5 - 0.5*veps*rstd^2
            nt = tmppool.tile([1, 16], F32, tag="nt")
            nc.vector.tensor_tensor(nt[:], veps[:], rstd[:], ALU.mult)
            nc.vector.tensor_tensor(nt[:], nt[:], rstd[:], ALU.mult)
            nc.scalar.activation(nt[:], nt[:], AF.Copy, bias=1.5, scale=-0.5)
            nc.vector.tensor_tensor(rstd[:], rstd[:], nt[:], ALU.mult)
            # A = rstd*gnw*(1-lam); Bc = (gnb - mean*rstd*gnw)*(1-lam)
            oml = tmppool.tile([1, 1], F32, tag="oml")
            nc.vector.tensor_scalar(oml[:], lam_sb[:], -1.0, 1.0, ALU.mult, ALU.add)
            A = tmppool.tile([1, 16], F32, tag="A")
            nc.vector.tensor_tensor(A[:], rstd[:], gnw_sb[:], ALU.mult)
            Bc = tmppool.tile([1, 16], F32, tag="Bc")
            nc.vector.tensor_tensor(Bc[:], mean[:], A[:], ALU.mult)
            nc.vector.tensor_tensor(Bc[:], gnb_sb[:], Bc[:], ALU.subtract)
            nc.vector.tensor_scalar_mul(A[:], A[:], oml[:, 0:1])
            nc.vector.tensor_scalar_mul(Bc[:], Bc[:], oml[:, 0:1])
            A_rep = tmppool.tile([128, 16], F32, tag="A_rep")
            nc.gpsimd.partition_broadcast(A_rep[:], A[:])

            # ================= post-collective combine =================
            # result = sum_h3 A[u,h3] * P[u][rh][h3] + (sum_h3 B[u,h3]*wsum[h3] + bo)
            for u in range(B):
                cb = tmppool.tile([1, 128], F32, tag="cb", name=f"cb_{u}")
                nc.vector.tensor_scalar_mul(
                    cb[:], wsum_sb[:, 0:128], Bc[:, 8 * u : 8 * u + 1]
                )
                for h3 in range(1, 8):
                    nc.vector.scalar_tensor_tensor(
                        cb[:], wsum_sb[:, 128 * h3 : 128 * (h3 + 1)],
                        Bc[:, 8 * u + h3 : 8 * u + h3 + 1], cb[:],
                        ALU.mult, ALU.add,
                    )
                nc.vector.tensor_tensor(cb[:], cb[:], bo_sb[:], ALU.add)
                cb_rep = tmppool.tile([128, 128], F32, tag="cb_rep", name=f"cbr_{u}")
                nc.gpsimd.partition_broadcast(cb_rep[:], cb[:])
                # res chunk rh: partition m'' holds row rho = 8*(m''%32) + 4*rh + m''//32
                for rh in range(2):
                    acc = tmppool.tile([128, 128], F32, tag="acc", name=f"acc_{u}_{rh}")
                    nc.vector.tensor_scalar_mul(
                        acc[:], P_sb[(u, rh, 0)][:], A_rep[:, 8 * u : 8 * u + 1]
                    )
                    for h3 in range(1, 8):
                        nc.vector.scalar_tensor_tensor(
                            acc[:], P_sb[(u, rh, h3)][:],
                            A_rep[:, 8 * u + h3 : 8 * u + h3 + 1], acc[:],
                            ALU.mult, ALU.add,
                        )
                    rsb = tmppool.tile([128, 128], F32, tag="rsb", name=f"rsb_{u}_{rh}")
                    nc.vector.tensor_tensor(rsb[:], acc[:], cb_rep[:], ALU.add)
                    dst = outp[u].rearrange("(b rh c) d -> rh c b d", b=32, rh=2, c=4)
                    nc.sync.dma_start(dst[rh], rsb[:])

    nc.compile()
    return nc


def _prep_inputs(inputs):
    """Host-side: slice/transpose full inputs into per-core in_maps."""
    query = np.asarray(inputs["query"], np.float32)
    Wq = np.asarray(inputs["Wq"], np.float32)
    Wk = np.asarray(inputs["Wk"], np.float32)
    Wv = np.asarray(inputs["Wv"], np.float32)
    Wo = np.asarray(inputs["Wo"], np.float32)
    bq = np.asarray(inputs["bq"], np.float32)
    bk = np.asarray(inputs["bk"], np.float32)
    bv = np.asarray(inputs["bv"], np.float32)
    bo = np.asarray(inputs["bo"], np.float32)
    gn_w = np.asarray(inputs["gn_w"], np.float32)
    gn_b = np.asarray(inputs["gn_b"], np.float32)
    lam = np.asarray(inputs["lam"], np.float32).reshape(1, 1)

    shared = {
        "wqT": np.ascontiguousarray(Wq.T),
        "wkT": np.ascontiguousarray(Wk.T),
        "wvT": np.ascontiguousarray(Wv.T),
        "woT": np.ascontiguousarray(Wo.T),
        "bqT": np.ascontiguousarray(bq.reshape(16, 128).T),
        "bkT": np.ascontiguousarray(bk.reshape(16, 128).T),
        "bv": bv.reshape(1, 1024),
        "bo": bo.reshape(1, 128),
        "gnw2": np.tile(gn_w, 2).reshape(1, 16),
        "gnb2": np.tile(gn_b, 2).reshape(1, 16),
        "lam": lam,
    }
    in_maps = []
    for c in range(N_CORES):
        blk = query[:, 256 * c : 256 * (c + 1), :]  # [B, 256, 128]
        qT = np.ascontiguousarray(blk.transpose(0, 2, 1))  # [B, 128, 256]
        in_maps.append({"qT": qT, **shared})
    return in_maps


class _Runner:
    """Cached-jit SPMD executor (one trace/compile; cheap repeated calls)."""

    def __init__(self, nc):
        import jax
        from jax.sharding import Mesh, PartitionSpec
        from jax.experimental.shard_map import shard_map
        from concourse.bass2jax import (
            install_neuronx_cc_hook, _bass_exec_p, partition_id_tensor,
        )

        install_neuronx_cc_hook()
        self.jax = jax
        pname = nc.partition_id_tensor.name if nc.partition_id_tensor else None
        in_names, out_names, out_avals, zero_outs = [], [], [], []
        for alloc in nc.m.functions[0].allocations:
            if not isinstance(alloc, mybir.MemoryLocationSet):
                continue
            name = alloc.memorylocations[0].name
            if alloc.kind == "ExternalInput":
                if name != pname:
                    in_names.append(name)
            elif alloc.kind == "ExternalOutput":
                out_names.append(name)
                shape = tuple(alloc.tensor_shape)
                dtype = mybir.dt.np(alloc.dtype)
                out_avals.append(jax.core.ShapedArray(shape, dtype))
                zero_outs.append(np.zeros(shape, dtype))
        self.in_names, self.out_names = in_names, out_names
        n_params = len(in_names)
        all_names = list(in_names) + out_names
        if pname is not None:
            all_names.append(pname)

        def _body(*args):
            operands = list(args)
            if pname is not None:
                operands.append(partition_id_tensor())
            return tuple(_bass_exec_p.bind(
                *operands, out_avals=tuple(out_avals), in_names=tuple(all_names),
                out_names=tuple(out_names), lowering_input_output_aliases=(),
                sim_require_finite=True, sim_require_nnan=True, nc=nc))

        devices = jax.devices()[:N_CORES]
        mesh = Mesh(np.asarray(devices), ("core",))
        nio = n_params + len(out_names)
        self.fn = jax.jit(
            shard_map(_body, mesh=mesh, in_specs=(PartitionSpec("core"),) * nio,
                      out_specs=(PartitionSpec("core"),) * len(out_names),
                      check_rep=False),
            keep_unused=True,
        )
        self.zeros = [
            jax.device_put(np.zeros((N_CORES * z.shape[0], *z.shape[1:]), z.dtype))
            for z in zero_outs
        ]
        self.out_shapes = [tuple(a.shape) for a in out_avals]

    def run(self, in_maps):
        cat = [
            np.concatenate([np.asarray(m[n]) for m in in_maps], axis=0)
            for n in self.in_names
        ]
        outs = self.fn(*cat, *self.zeros)
        self.jax.block_until_ready(outs)
        return [
            {n: np.asarray(outs[i]).reshape(N_CORES, *self.out_shapes[i])[c]
             for i, n in enumerate(self.out_names)}
            for c in range(N_CORES)
        ]


_CACHED_NC = None


def kernel(**inputs) -> np.ndarray:
    global _CACHED, _CACHED_NC
    if _CACHED is None:
        _CACHED_NC = build_nc()
        _CACHED = _Runner(_CACHED_NC)
    in_maps = _prep_inputs(inputs)
    results = _CACHED.run(in_maps)
    out = np.empty((B, S, H * D // 8), np.float32)  # (2, 2048, 128)
    for c in range(N_CORES):
        o = results[c]["outp"]  # [B, 256, 128]
        for b in range(B):
            out[b, c::8, :] = o[b]  # rows s3 = 8*rho + c
    return out


# revision 28
# speedup vs baseline: 3840.3930x; 1.1103x over previous
"""DiffAttention Trainium2 kernel, 8-core SPMD (head-parallel).

Problem (hardcoded): B=2, S=2048, D=128, H=8.
  q = (x@Wq.T+bq).reshape(B,H,S,2D)   # raw reshape: head h <-> rows [256h,256h+256) of proj
  s1 = q1@k1.T; s2 = q2@k2.T; attn = softmax(s1) - lam*softmax(s2)
  out = attn@v -> transpose/reshape -> GroupNorm(H groups) -> *(1-lam) -> concat heads -> @Wo.T+bo

Sharding: core c owns head h=c for both batches (2 units/core). GroupNorm groups
mix all heads -> tiny (32-float) AllGather of partial stats.

Index algebra per unit (b,h), block = proj rows [256h, 256h+256):
  sigma (attn row) = 8r+j, r in [0,256), j in [0,8). We use tau-order sigma' = 256j+r.
  q1T[d, sigma'=256j+r] = qpT_block[f=256j+d, r]   (even 128-col chunks of qp block)
  q2T: odd chunks.  v'[sigma'=256j+r, d] = vp_block[r, 128j+d].
  GroupNorm group g = {sigma': (sigma' mod 256)//32 == g} (32-wide strips).
  Final rows: out[b, 8*rho+h, 128h3+d] = GN(O)[b,h][sigma'=256(rho%8)+32h3+rho//8, d]
"""

import sys

sys.path.insert(0, "/opt/trn_rl_repo")

import numpy as np

import concourse.bass as bass
import concourse.bacc as bacc
import concourse.mybir as mybir
import concourse.tile as tile
from concourse.bass_utils import run_bass_kernel_spmd

F32 = mybir.dt.float32
F32R = mybir.dt.float32r
AF = mybir.ActivationFunctionType
ALU = mybir.AluOpType

B, S, D, H = 2, 2048, 128, 8
N_CORES = 8
EPS = 1e-5
GROUP_N = float(256 * H * D)  # elements per GroupNorm group

_CACHED = None


def build_nc():
    nc = bacc.Bacc("TRN2", target_bir_lowering=False, debug=False, num_devices=N_CORES)

    # ---- per-core external I/O ----
    qT = nc.dram_tensor("qT", [B, 128, 256], F32, kind="ExternalInput")  # query block.T per batch
    wqT = nc.dram_tensor("wqT", [128, 2048], F32, kind="ExternalInput")
    wkT = nc.dram_tensor("wkT", [128, 2048], F32, kind="ExternalInput")
    wvT = nc.dram_tensor("wvT", [128, 1024], F32, kind="ExternalInput")
    woT = nc.dram_tensor("woT", [1024, 128], F32, kind="ExternalInput")
    bqT = nc.dram_tensor("bqT", [128, 16], F32, kind="ExternalInput")
    bkT = nc.dram_tensor("bkT", [128, 16], F32, kind="ExternalInput")
    bv = nc.dram_tensor("bv", [1, 1024], F32, kind="ExternalInput")
    bo = nc.dram_tensor("bo", [1, 128], F32, kind="ExternalInput")
    gnw2 = nc.dram_tensor("gnw2", [1, 16], F32, kind="ExternalInput")  # tiled x2 (b,g)
    gnb2 = nc.dram_tensor("gnb2", [1, 16], F32, kind="ExternalInput")
    lam = nc.dram_tensor("lam", [1, 1], F32, kind="ExternalInput")
    outp = nc.dram_tensor("outp", [B, 256, 128], F32, kind="ExternalOutput")

    with tile.TileContext(nc) as tc:
        with (
            tc.tile_pool(name="const", bufs=1) as cpool,
            tc.tile_pool(name="proj", bufs=2) as projpool,
            tc.tile_pool(name="vpool", bufs=4) as vpool,
            tc.tile_pool(name="epool", bufs=3) as epool,
            tc.tile_pool(name="otpool", bufs=2) as otpool,
            tc.tile_pool(name="tmp", bufs=2) as tmppool,
            tc.tile_pool(name="ps_s", bufs=2, space="PSUM") as ps_s,
            tc.tile_pool(name="ps_acc", bufs=1, space="PSUM") as ps_acc,
            tc.tile_pool(name="dram", bufs=1, space="DRAM") as dram,
        ):
            # ---- load constants / weights (qT first: projections need it) ----
            qt_sb = []
            for u in range(B):
                q = cpool.tile([128, 256], F32, name=f"qt_sb{u}")
                nc.sync.dma_start(q[:], qT[u])
                qt_sb.append(q)

            # small constants go on the gpsimd DMA queue so they don't delay
            # the big weight DMAs on the sync queue
            bq_sb = cpool.tile([128, 16], F32)
            bk_sb = cpool.tile([128, 16], F32)
            nc.gpsimd.dma_start(bq_sb[:], bqT[:])
            nc.gpsimd.dma_start(bk_sb[:], bkT[:])
            bv_sb = cpool.tile([1, 1024], F32)
            nc.gpsimd.dma_start(bv_sb[:], bv[:])
            bo_sb = cpool.tile([1, 128], F32)
            nc.gpsimd.dma_start(bo_sb[:], bo[:])
            gnw_sb = cpool.tile([1, 16], F32)
            gnb_sb = cpool.tile([1, 16], F32)
            nc.gpsimd.dma_start(gnw_sb[:], gnw2[:])
            nc.gpsimd.dma_start(gnb_sb[:], gnb2[:])
            lam_sb = cpool.tile([1, 1], F32)
            nc.gpsimd.dma_start(lam_sb[:], lam[:])
            ones_f32 = cpool.tile([128, 128], F32)
            nc.vector.memset(ones_f32[:], 1.0)
            ones_sb = cpool.tile([128, 128], F32R)
            nc.vector.tensor_copy(ones_sb[:], ones_f32[:])
            # weights loaded and f32r-rounded in 1024-col pieces so projections
            # (and then attention) can start before all input DMA completes.
            wq_rh, wk_rh = [], []
            wv_r = cpool.tile([128, 1024], F32R)
            wpieces = (
                [("wq", wqT, wq_rh, 0), ("wq", wqT, wq_rh, 1),
                 ("wv", wvT, None, 0),
                 ("wk", wkT, wk_rh, 0), ("wk", wkT, wk_rh, 1)]
            )
            for (wnm, dram_w, lst, half) in wpieces:
                wsc = projpool.tile([128, 1024], F32, tag="wsc", name=f"wsc_{wnm}{half}")
                nc.sync.dma_start(wsc[:], dram_w[:, 1024 * half : 1024 * (half + 1)])
                if lst is None:
                    nc.vector.tensor_copy(wv_r[:], wsc[:])
                else:
                    wr = cpool.tile([128, 1024], F32R, name=f"{wnm}_r{half}")
                    nc.vector.tensor_copy(wr[:], wsc[:])
                    lst.append(wr)
            qt_r = []
            for u in range(B):
                qr = cpool.tile([128, 256], F32R, name=f"qt_r{u}")
                nc.vector.tensor_copy(qr[:], qt_sb[u][:])
                qt_r.append(qr)
            lam_rep = cpool.tile([128, 1], F32)
            nc.gpsimd.partition_broadcast(lam_rep[:], lam_sb[:])
            bv_rep = cpool.tile([128, 1024], F32)
            nc.gpsimd.partition_broadcast(bv_rep[:], bv_sb[:])
            bo_rep = cpool.tile([128, 128], F32)
            nc.gpsimd.partition_broadcast(bo_rep[:], bo_sb[:])
            # Wo chunks + their column sums (needed only in the output phase)
            wo_sb = []
            for h3 in range(8):
                w = cpool.tile([128, 128], F32, name=f"wo_sb{h3}")
                nc.sync.dma_start(w[:], woT[128 * h3 : 128 * (h3 + 1), :])
                wo_sb.append(w)
            wsum_sb = cpool.tile([1, 1024], F32)
            for h3 in range(8):
                wps = ps_s.tile([1, 128], F32, tag="s", name=f"wps_{h3}")
                nc.tensor.matmul(wps[:], ones_f32[:, 0:1], wo_sb[h3][:], start=True, stop=True)
                nc.vector.tensor_copy(wsum_sb[:, 128 * h3 : 128 * (h3 + 1)], wps[:])

            # ================= projections (both units) =================
            stats_sb = tmppool.tile([1, 32], F32, tag="stats")
            p2_tiles = []
            fT_sb = []  # re-laid-out OT per unit: fT[d, 256g+32j+r] = OT[d, 256j+32g+r]
            proj = []  # (q1t, q2t, k1t, k2t, vp) per unit
            # q1t/q2t as four [128,512] qb-quarter tiles, k1t/k2t as two
            # [128,1024] halves: fine-grained tiles let attention start as soon
            # as the first pieces are projected.
            qk = {}
            for u in range(B):
                for nm in ("q1", "q2"):
                    qk[(u, nm)] = [
                        projpool.tile([128, 512], F32R, tag=f"{nm}t",
                                      name=f"{nm}t_{u}_{qb}", bufs=8)
                        for qb in range(4)
                    ]
                for nm in ("k1", "k2"):
                    qk[(u, nm)] = [
                        projpool.tile([128, 1024], F32R, tag=f"{nm}t",
                                      name=f"{nm}t_{u}_{hh}", bufs=4)
                        for hh in range(2)
                    ]
            # phase 1: q projections (both units) -- needs only wq
            for u in range(B):
                for j in range(16):
                    ps = ps_s.tile([128, 256], F32, tag="s", name=f"pp_{u}_{j}")
                    nc.tensor.matmul(
                        ps[:], wq_rh[j // 8][:, 128 * (j % 8) : 128 * (j % 8 + 1)],
                        qt_r[u][:], start=True, stop=True,
                    )
                    dst = qk[(u, "q1" if j % 2 == 0 else "q2")][j // 4]
                    col = 256 * ((j // 2) % 2)
                    nc.vector.tensor_scalar_add(
                        dst[:, col : col + 256], ps[:], bq_sb[:, j : j + 1]
                    )
            # phase 2: v projections
            vps = []
            for u in range(B):
                vp = []
                for rc in range(2):
                    vt = vpool.tile([128, 1024], F32R, tag="vp", name=f"vp_{u}_{rc}")
                    for fh in range(2):
                        ps = ps_s.tile([128, 512], F32, tag="s", name=f"ppv_{u}_{rc}_{fh}")
                        nc.tensor.matmul(
                            ps[:], qt_r[u][:, 128 * rc : 128 * (rc + 1)],
                            wv_r[:, 512 * fh : 512 * (fh + 1)],
                            start=True, stop=True,
                        )
                        nc.vector.tensor_tensor(
                            vt[:, 512 * fh : 512 * (fh + 1)], ps[:],
                            bv_rep[:, 512 * fh : 512 * (fh + 1)], ALU.add,
                        )
                    vp.append(vt)
                vps.append(vp)
            # phase 3: k projections
            for u in range(B):
                for j in range(16):
                    ps = ps_s.tile([128, 256], F32, tag="s", name=f"pk_{u}_{j}")
                    nc.tensor.matmul(
                        ps[:], wk_rh[j // 8][:, 128 * (j % 8) : 128 * (j % 8 + 1)],
                        qt_r[u][:], start=True, stop=True,
                    )
                    dst = qk[(u, "k1" if j % 2 == 0 else "k2")][j // 8]
                    col = 256 * ((j // 2) % 4)
                    nc.vector.tensor_scalar_add(
                        dst[:, col : col + 256], ps[:], bk_sb[:, j : j + 1]
                    )
            for u in range(B):
                proj.append((qk[(u, "q1")], qk[(u, "q2")],
                             qk[(u, "k1")], qk[(u, "k2")], vps[u]))

            # ================= attention + stats (both units) =================
            for u in range(B):
                q1l, q2l, k1l, k2l, vp = proj[u]

                def vchunk(kc):
                    # v' chunk kc: [128 k, 128 d] = vp[kc%2][:, 128*(kc//2):+128]
                    return vp[kc % 2][:, 128 * (kc // 2) : 128 * (kc // 2) + 128]

                fT = tmppool.tile([128, 2048], F32, tag="sq", name=f"fT_{u}")
                fT_sb.append(fT)
                fv4 = fT.rearrange("p (g j r) -> p g j r", g=8, j=8, r=32)
                p1a = tmppool.tile([128, 16], F32, tag="p1a", name=f"p1a_{u}")
                p1b = tmppool.tile([128, 16], F32, tag="p1b", name=f"p1b_{u}")

                for qb in range(4):
                    qsl = slice(512 * qb, 512 * (qb + 1))
                    u1 = ps_acc.tile([128, 512], F32, tag="u1", name=f"u1_{u}_{qb}")
                    u2 = ps_acc.tile([128, 512], F32, tag="u2", name=f"u2_{u}_{qb}")
                    r1 = ps_acc.tile([128, 512], F32, tag="r1", name=f"r1_{u}_{qb}")
                    r2 = ps_acc.tile([128, 512], F32, tag="r2", name=f"r2_{u}_{qb}")
                    def consume(item):
                        kcg, eg, uacc, racc = item
                        for h in range(2):
                            kc = 2 * kcg + h
                            esl = eg[:, 512 * h : 512 * (h + 1)]
                            nc.tensor.matmul(
                                racc[:], ones_sb[:], esl,
                                start=(kcg == 0 and h == 0),
                                stop=(kcg == 7 and h == 1),
                            )
                            nc.tensor.matmul(
                                uacc[:], vchunk(kc), esl,
                                start=(kcg == 0 and h == 0),
                                stop=(kcg == 7 and h == 1),
                            )

                    pending = None
                    for kcg in range(8):
                        for m, (kl, qtile, uacc, racc) in enumerate(
                            ((k1l, q1l[qb], u1, r1), (k2l, q2l[qb], u2, r2))
                        ):
                            sgrp = ps_s.tile([128, 1024], F32, tag="s", name=f"s_{u}_{qb}_{kcg}_{m}")
                            for h in range(2):
                                kc = 2 * kcg + h
                                nc.tensor.matmul(
                                    sgrp[:, 512 * h : 512 * (h + 1)],
                                    kl[kc // 8][:, 128 * (kc % 8) : 128 * (kc % 8 + 1)],
                                    qtile[:],
                                    start=True, stop=True,
                                )
                            eg = epool.tile([128, 1024], F32R, tag="e", name=f"e_{u}_{qb}_{kcg}_{m}")
                            nc.scalar.activation(eg[:], sgrp[:], AF.Exp)
                            if pending is not None:
                                consume(pending)
                            pending = (kcg, eg, uacc, racc)
                    consume(pending)
                    # O = U1/R1 - lam*U2/R2   (R replicated across partitions)
                    r1i = tmppool.tile([128, 512], F32, tag="r1i", name=f"r1i_{u}_{qb}")
                    r2i = tmppool.tile([128, 512], F32, tag="r2i", name=f"r2i_{u}_{qb}")
                    nc.vector.reciprocal(r1i[:], r1[:])
                    nc.vector.reciprocal(r2i[:], r2[:])
                    t2 = tmppool.tile([128, 512], F32, tag="t2", name=f"t2_{u}_{qb}")
                    nc.vector.scalar_tensor_tensor(
                        t2[:], u2[:], lam_rep[:, 0:1], r2i[:], ALU.mult, ALU.mult
                    )
                    t1 = tmppool.tile([128, 512], F32, tag="t1", name=f"t1_{u}_{qb}")
                    nc.vector.tensor_tensor(t1[:], u1[:], r1i[:], ALU.mult)
                    otq = otpool.tile([128, 512], F32, tag="ot", name=f"ot_{u}_{qb}")
                    nc.vector.tensor_tensor(otq[:], t1[:], t2[:], ALU.subtract)

                    # incremental GroupNorm partial stats for this q-block
                    # (free-dim layout within the block: (j2, g8, r32))
                    osl = otq.rearrange("p (j g r) -> p j g r", j=2, g=8, r=32)
                    red = tmppool.tile([128, 16], F32, tag="red", name=f"red_{u}_{qb}")
                    nc.vector.tensor_reduce(red[:], osl, mybir.AxisListType.X, ALU.add)
                    if qb == 0:
                        nc.vector.tensor_copy(p1a[:], red[:])
                    else:
                        nc.vector.tensor_tensor(p1a[:], p1a[:], red[:], ALU.add)
                    sq5 = tmppool.tile([128, 512], F32, tag="t1", name=f"sq5_{u}_{qb}")
                    nc.vector.tensor_tensor(sq5[:], otq[:], otq[:], ALU.mult)
                    redb = tmppool.tile([128, 16], F32, tag="redb", name=f"redb_{u}_{qb}")
                    nc.vector.tensor_reduce(
                        redb[:], sq5.rearrange("p (j g r) -> p j g r", j=2, g=8, r=32),
                        mybir.AxisListType.X, ALU.add,
                    )
                    if qb == 0:
                        nc.vector.tensor_copy(p1b[:], redb[:])
                    else:
                        nc.vector.tensor_tensor(p1b[:], p1b[:], redb[:], ALU.add)
                    # incremental re-layout into fT (j-pair slab for this qb);
                    # src re-viewed g-outer to match the dst iteration order
                    nc.vector.tensor_copy(
                        fv4[:, :, 2 * qb : 2 * qb + 2, :],
                        otq.rearrange("p (j g r) -> p g j r", j=2, g=8, r=32),
                    )

                # fold (j mod 2) pairs -> per-group partials
                for si, p1x in enumerate((p1a, p1b)):
                    p2 = tmppool.tile([128, 8], F32, tag="p2", name=f"p2_{u}_{si}")
                    nc.vector.tensor_reduce(
                        p2[:], p1x.rearrange("p (j g) -> p g j", j=2, g=8),
                        mybir.AxisListType.X, ALU.add,
                    )
                    p2_tiles.append((u, si, p2))

            # deferred partition-sums of the per-unit stats
            for (u, si, p2) in p2_tiles:
                st = ps_s.tile([1, 8], F32, tag="s", name=f"st_{u}_{si}")
                nc.tensor.matmul(st[:], ones_f32[:, 0:1], p2[:], start=True, stop=True)
                nc.vector.tensor_copy(
                    stats_sb[:, 16 * si + 8 * u : 16 * si + 8 * u + 8], st[:]
                )

            # ================= AllGather of per-unit stats =================
            cc_in = dram.tile([1, 32], F32)
            cc_out = dram.tile([8, 32], F32, addr_space="Shared")
            nc.sync.dma_start(cc_in[:], stats_sb[:])
            nc.gpsimd.collective_compute(
                "AllGather", ALU.bypass,
                replica_groups=[list(range(N_CORES))],
                ins=[cc_in[:]], outs=[cc_out[:]],
            )

            # -- overlapped with the collective: unnormalized per-h3 partial
            #    output matmuls P[u][rh][h3] (fT was filled during attention) --
            P_sb = {}
            for u in range(B):
                fT = fT_sb[u]
                for rh in range(2):
                    for h3 in range(8):
                        pps = ps_s.tile([128, 128], F32, tag="s", name=f"pps_{u}_{rh}_{h3}")
                        lhsT = fT[:, 256 * h3 + 128 * rh : 256 * h3 + 128 * rh + 128]
                        nc.tensor.matmul(pps[:], lhsT, wo_sb[h3][:], start=True, stop=True)
                        pt = cpool.tile([128, 128], F32, name=f"P_{u}_{rh}_{h3}")
                        nc.vector.tensor_copy(pt[:], pps[:])
                        P_sb[(u, rh, h3)] = pt

            gath = tmppool.tile([8, 32], F32, tag="gath")
            nc.sync.dma_start(gath[:], cc_out[:])
            glob_ps = ps_s.tile([1, 32], F32, tag="s", name="glob_ps")
            nc.tensor.matmul(glob_ps[:], ones_f32[0:8, 0:1], gath[:], start=True, stop=True)
            glob = tmppool.tile([1, 32], F32, tag="globsb")
            nc.vector.tensor_copy(glob[:], glob_ps[:])
            # [1,32] layout: [sum b0(8) | sum b1(8) | sq b0(8) | sq b1(8)]
            sums = glob[:, 0:16]
            sqs = glob[:, 16:32]
            mean = tmppool.tile([1, 16], F32, tag="mean")
            ex2 = tmppool.tile([1, 16], F32, tag="ex2")
            nc.vector.tensor_scalar_mul(mean[:], sums, 1.0 / GROUP_N)
            nc.vector.tensor_scalar_mul(ex2[:], sqs, 1.0 / GROUP_N)
            var = tmppool.tile([1, 16], F32, tag="var")
            nc.vector.tensor_tensor(var[:], mean[:], mean[:], ALU.mult)
            nc.vector.tensor_tensor(var[:], ex2[:], var[:], ALU.subtract)
            veps = tmppool.tile([1, 16], F32, tag="veps")
            nc.vector.tensor_scalar_add(veps[:], var[:], EPS)
            rstd = tmppool.tile([1, 16], F32, tag="rstd")
            vinv = tmppool.tile([1, 16], F32, tag="vinv")
            nc.vector.reciprocal(vinv[:], veps[:])
            nc.scalar.activation(rstd[:], vinv[:], AF.Sqrt)
            # one Newton step: rstd *= 1.5 - 0.5*veps*rstd^2
            nt = tmppool.tile([1, 16], F32, tag="nt")
            nc.vector.tensor_tensor(nt[:], veps[:], rstd[:], ALU.mult)
            nc.vector.tensor_tensor(nt[:], nt[:], rstd[:], ALU.mult)
            nc.scalar.activation(nt[:], nt[:], AF.Copy, bias=1.5, scale=-0.5)
            nc.vector.tensor_tensor(rstd[:], rstd[:], nt[:], ALU.mult)
            # A = rstd*gnw*(1-lam); Bc = (gnb - mean*rstd*gnw)*(1-lam)
            oml = tmppool.tile([1, 1], F32, tag="oml")
            nc.vector.tensor_scalar(oml[:], lam_sb[:], -1.0, 1.0, ALU.mult, ALU.add)
            A = tmppool.tile([1, 16], F32, tag="A")
            nc.vector.tensor_tensor(A[:], rstd[:], gnw_sb[:], ALU.mult)
            Bc = tmppool.tile([1, 16], F32, tag="Bc")
            nc.vector.tensor_tensor(Bc[:], mean[:], A[:], ALU.mult)
            nc.vector.tensor_tensor(Bc[:], gnb_sb[:], Bc[:], ALU.subtract)
            nc.vector.tensor_scalar_mul(A[:], A[:], oml[:, 0:1])
            nc.vector.tensor_scalar_mul(Bc[:], Bc[:], oml[:, 0:1])
            A_rep = tmppool.tile([128, 16], F32, tag="A_rep")
            nc.gpsimd.partition_broadcast(A_rep[:], A[:])

            # ================= post-collective combine =================
            # result = sum_h3 A[u,h3] * P[u][rh][h3] + (sum_h3 B[u,h3]*wsum[h3] + bo)
            for u in range(B):
                cb = tmppool.tile([1, 128], F32, tag="cb", name=f"cb_{u}")
                nc.vector.tensor_scalar_mul(
                    cb[:], wsum_sb[:, 0:128], Bc[:, 8 * u : 8 * u + 1]
                )
                for h3 in range(1, 8):
                    nc.vector.scalar_tensor_tensor(
                        cb[:], wsum_sb[:, 128 * h3 : 128 * (h3 + 1)],
                        Bc[:, 8 * u + h3 : 8 * u + h3 + 1], cb[:],
                        ALU.mult, ALU.add,
                    )
                nc.vector.tensor_tensor(cb[:], cb[:], bo_sb[:], ALU.add)
                cb_rep = tmppool.tile([128, 128], F32, tag="cb_rep", name=f"cbr_{u}")
                nc.gpsimd.partition_broadcast(cb_rep[:], cb[:])
                # res chunk rh: partition m'' holds row rho = 8*(m''%32) + 4*rh + m''//32
                for rh in range(2):
                    acc = tmppool.tile([128, 128], F32, tag="acc", name=f"acc_{u}_{rh}")
                    nc.vector.tensor_scalar_mul(
                        acc[:], P_sb[(u, rh, 0)][:], A_rep[:, 8 * u : 8 * u + 1]
                    )
                    for h3 in range(1, 8):
                        nc.vector.scalar_tensor_tensor(
                            acc[:], P_sb[(u, rh, h3)][:],
                            A_rep[:, 8 * u + h3 : 8 * u + h3 + 1], acc[:],
                            ALU.mult, ALU.add,
                        )
                    rsb = tmppool.tile([128, 128], F32, tag="rsb", name=f"rsb_{u}_{rh}")
                    nc.vector.tensor_tensor(rsb[:], acc[:], cb_rep[:], ALU.add)
                    # contiguous block write; host undoes the row permutation
                    # (device row 128*rh+m'' holds rho = 8*(m''%32)+4*rh+m''//32)
                    nc.sync.dma_start(outp[u][128 * rh : 128 * (rh + 1), :], rsb[:])

    nc.compile()
    return nc


def _prep_inputs(inputs):
    """Host-side: slice/transpose full inputs into per-core in_maps."""
    query = np.asarray(inputs["query"], np.float32)
    Wq = np.asarray(inputs["Wq"], np.float32)
    Wk = np.asarray(inputs["Wk"], np.float32)
    Wv = np.asarray(inputs["Wv"], np.float32)
    Wo = np.asarray(inputs["Wo"], np.float32)
    bq = np.asarray(inputs["bq"], np.float32)
    bk = np.asarray(inputs["bk"], np.float32)
    bv = np.asarray(inputs["bv"], np.float32)
    bo = np.asarray(inputs["bo"], np.float32)
    gn_w = np.asarray(inputs["gn_w"], np.float32)
    gn_b = np.asarray(inputs["gn_b"], np.float32)
    lam = np.asarray(inputs["lam"], np.float32).reshape(1, 1)

    shared = {
        "wqT": np.ascontiguousarray(Wq.T),
        "wkT": np.ascontiguousarray(Wk.T),
        "wvT": np.ascontiguousarray(Wv.T),
        "woT": np.ascontiguousarray(Wo.T),
        "bqT": np.ascontiguousarray(bq.reshape(16, 128).T),
        "bkT": np.ascontiguousarray(bk.reshape(16, 128).T),
        "bv": bv.reshape(1, 1024),
        "bo": bo.reshape(1, 128),
        "gnw2": np.tile(gn_w, 2).reshape(1, 16),
        "gnb2": np.tile(gn_b, 2).reshape(1, 16),
        "lam": lam,
    }
    in_maps = []
    for c in range(N_CORES):
        blk = query[:, 256 * c : 256 * (c + 1), :]  # [B, 256, 128]
        qT = np.ascontiguousarray(blk.transpose(0, 2, 1))  # [B, 128, 256]
        in_maps.append({"qT": qT, **shared})
    return in_maps


class _Runner:
    """Cached-jit SPMD executor (one trace/compile; cheap repeated calls)."""

    def __init__(self, nc):
        import jax
        from jax.sharding import Mesh, PartitionSpec
        from jax.experimental.shard_map import shard_map
        from concourse.bass2jax import (
            install_neuronx_cc_hook, _bass_exec_p, partition_id_tensor,
        )

        install_neuronx_cc_hook()
        self.jax = jax
        pname = nc.partition_id_tensor.name if nc.partition_id_tensor else None
        in_names, out_names, out_avals, zero_outs = [], [], [], []
        for alloc in nc.m.functions[0].allocations:
            if not isinstance(alloc, mybir.MemoryLocationSet):
                continue
            name = alloc.memorylocations[0].name
            if alloc.kind == "ExternalInput":
                if name != pname:
                    in_names.append(name)
            elif alloc.kind == "ExternalOutput":
                out_names.append(name)
                shape = tuple(alloc.tensor_shape)
                dtype = mybir.dt.np(alloc.dtype)
                out_avals.append(jax.core.ShapedArray(shape, dtype))
                zero_outs.append(np.zeros(shape, dtype))
        self.in_names, self.out_names = in_names, out_names
        n_params = len(in_names)
        all_names = list(in_names) + out_names
        if pname is not None:
            all_names.append(pname)

        def _body(*args):
            operands = list(args)
            if pname is not None:
                operands.append(partition_id_tensor())
            return tuple(_bass_exec_p.bind(
                *operands, out_avals=tuple(out_avals), in_names=tuple(all_names),
                out_names=tuple(out_names), lowering_input_output_aliases=(),
                sim_require_finite=True, sim_require_nnan=True, nc=nc))

        devices = jax.devices()[:N_CORES]
        mesh = Mesh(np.asarray(devices), ("core",))
        nio = n_params + len(out_names)
        self.fn = jax.jit(
            shard_map(_body, mesh=mesh, in_specs=(PartitionSpec("core"),) * nio,
                      out_specs=(PartitionSpec("core"),) * len(out_names),
                      check_rep=False),
            keep_unused=True,
        )
        self.zeros = [
            jax.device_put(np.zeros((N_CORES * z.shape[0], *z.shape[1:]), z.dtype))
            for z in zero_outs
        ]
        self.out_shapes = [tuple(a.shape) for a in out_avals]

    def run(self, in_maps):
        cat = [
            np.concatenate([np.asarray(m[n]) for m in in_maps], axis=0)
            for n in self.in_names
        ]
        outs = self.fn(*cat, *self.zeros)
        self.jax.block_until_ready(outs)
        return [
            {n: np.asarray(outs[i]).reshape(N_CORES, *self.out_shapes[i])[c]
             for i, n in enumerate(self.out_names)}
            for c in range(N_CORES)
        ]


_CACHED_NC = None


def kernel(**inputs) -> np.ndarray:
    global _CACHED, _CACHED_NC
    if _CACHED is None:
        _CACHED_NC = build_nc()
        _CACHED = _Runner(_CACHED_NC)
    in_maps = _prep_inputs(inputs)
    results = _CACHED.run(in_maps)
    # device row (rh, m'') holds output row rho = 8*(m'' % 32) + 4*rh + m''//32
    mpp = np.arange(128)
    rho = np.concatenate([8 * (mpp % 32) + 4 * rh + mpp // 32 for rh in (0, 1)])
    inv = np.argsort(rho)
    out = np.empty((B, S, H * D // 8), np.float32)  # (2, 2048, 128)
    for c in range(N_CORES):
        o = results[c]["outp"]  # [B, 256, 128] in device (rh, m'') row order
        for b in range(B):
            out[b, c::8, :] = o[b][inv]  # rows s3 = 8*rho + c
    return out
